# Initial kernel scaffold
#
"""Segment mean-pool (LocalPooling1D) Trainium2 Bass kernel.

x [32, 8192, 256] f32, x_pos [32, 65] sorted int32 boundaries -> y [32, 64, 256].
y[b, j] = mean(x[b, x_pos[b,j]:x_pos[b,j+1]]), empty segments -> 0.

Strategy: data-parallel over batch, 4 rows per core on 8 cores. Per 128-token
tile, build the 0/1 segment-indicator [128 tok, 64 seg] on-chip from x_pos
(step-function difference: S[t,j] = (pos[j] <= t), ind = S[:, :64] - S[:, 1:])
and accumulate psum[64, 256] += ind.T @ x_tile on the TensorEngine. Divide by
clamped segment counts at the end.
"""

import os
import sys

import numpy as np

sys.path.insert(0, "/opt/trn_rl_repo")

import concourse.bacc as bacc
import concourse.bass as bass
import concourse.tile as tile
from concourse import mybir
from concourse.bass_utils import run_bass_kernel_spmd

dt = mybir.dt
Alu = mybir.AluOpType

# Problem constants (hardcoded per harness contract).
B, T, C, P = 32, 8192, 256, 65
NSEG = P - 1
NCORES = 8
R = B // NCORES          # batch rows per core
TOK = 128                # tokens per matmul tile (K)
KTILES = T // TOK        # 64 matmul tiles per row

# Tunables (env-overridable for experiments).
CFG = {
    "blk": int(os.environ.get("KB_BLK", "8")),            # token-tiles per x DMA
    "mm_dtype": os.environ.get("KB_MMDT", "f32r"),        # f32 | f32r
    "use_act_sign": os.environ.get("KB_ACT", "1") == "1", # S on ScalarE via Sign
    "x_bufs": int(os.environ.get("KB_XBUFS", "4")),
    "ind_bufs": int(os.environ.get("KB_INDBUFS", "8")),
    "psum_bufs": int(os.environ.get("KB_PSUMBUFS", "2")),
}


def build_program(cfg=CFG):
    blk = cfg["blk"]
    nblk = KTILES // blk
    mm_dt = dt.float32r if cfg["mm_dtype"] == "f32r" else dt.float32
    use_act = cfg["use_act_sign"]

    nc = bacc.Bacc("TRN2", target_bir_lowering=False, debug=False)

    x_d = nc.dram_tensor("x", [R, T, C], dt.float32, kind="ExternalInput")
    pos_d = nc.dram_tensor("x_pos", [R, P], dt.int32, kind="ExternalInput")
    y_d = nc.dram_tensor("y", [R, NSEG, C], dt.float32, kind="ExternalOutput")

    with tile.TileContext(nc) as tc:
        with (
            tc.tile_pool(name="const", bufs=1) as constp,
            tc.tile_pool(name="xp", bufs=cfg["x_bufs"]) as xp,
            tc.tile_pool(name="sp", bufs=cfg["ind_bufs"]) as sp,
            tc.tile_pool(name="indp", bufs=cfg["ind_bufs"]) as indp,
            tc.tile_pool(name="smallp", bufs=4) as smallp,
            tc.tile_pool(name="outp", bufs=2) as outp,
            tc.tile_pool(name="psp", bufs=cfg["psum_bufs"], space="PSUM") as psp,
        ):
            # t_global per (partition, tile): val = p + 128*ti.
            tg_i = constp.tile([TOK, KTILES], dt.int32)
            nc.gpsimd.iota(tg_i[:], pattern=[[TOK, KTILES]], base=0, channel_multiplier=1)
            tgf = constp.tile([TOK, KTILES], dt.float32)
            if use_act:
                # sign(t + 0.5 - pos) trick needs the half-offset baked in.
                nc.vector.tensor_scalar(tgf[:], tg_i[:], 0.5, None, op0=Alu.add)
            else:
                nc.vector.tensor_copy(tgf[:], tg_i[:])

            for r in range(R):
                # pos row -> [1, 65] f32, broadcast to [128, 65].
                pos_row = smallp.tile([1, P], dt.int32)
                nc.sync.dma_start(pos_row[:], pos_d[r : r + 1, :])
                posf_row = smallp.tile([1, P], dt.float32)
                nc.vector.tensor_copy(posf_row[:], pos_row[:])
                pos_b = smallp.tile([TOK, P], dt.float32)
                nc.gpsimd.partition_broadcast(pos_b[:], posf_row[:])

                # counts -> 1/max(cnt, 1), partition-major [64, 1].
                pos_lo = smallp.tile([NSEG, 1], dt.int32)
                pos_hi = smallp.tile([NSEG, 1], dt.int32)
                nc.sync.dma_start(pos_lo[:], pos_d[r : r + 1, 0:NSEG].rearrange("one p -> p one"))
                nc.sync.dma_start(pos_hi[:], pos_d[r : r + 1, 1:P].rearrange("one p -> p one"))
                cnt_f = smallp.tile([NSEG, 1], dt.float32)
                nc.vector.tensor_tensor(cnt_f[:], pos_hi[:], pos_lo[:], op=Alu.subtract)
                cntc = smallp.tile([NSEG, 1], dt.float32)
                if use_act:
                    # ind comes out as 2*indicator; fold the 1/2 into the recip.
                    nc.vector.tensor_scalar(cntc[:], cnt_f[:], 1.0, 2.0, op0=Alu.max, op1=Alu.mult)
                else:
                    nc.vector.tensor_scalar(cntc[:], cnt_f[:], 1.0, None, op0=Alu.max)
                recip = smallp.tile([NSEG, 1], dt.float32)
                nc.vector.reciprocal(recip[:], cntc[:])

                ps = psp.tile([NSEG, C], dt.float32)
                xr = x_d[r].rearrange("(b k p) c -> b p (k c)", k=blk, p=TOK)
                for b in range(nblk):
                    xt = xp.tile([TOK, blk * C], mm_dt)
                    nc.sync.dma_start(xt[:], xr[b])
                    for k in range(blk):
                        ti = b * blk + k
                        S = sp.tile([TOK, P], dt.float32)
                        if use_act:
                            # S' = sign(t + 0.5 - pos[j]) in {-1, +1} = 2S - 1
                            nc.scalar.activation(
                                S[:], pos_b[:], mybir.ActivationFunctionType.Sign,
                                bias=tgf[:, ti : ti + 1], scale=-1.0,
                            )
                        else:
                            nc.vector.tensor_scalar(
                                S[:], pos_b[:], tgf[:, ti : ti + 1], None, op0=Alu.is_le
                            )
                        ind = indp.tile([TOK, NSEG], mm_dt)
                        nc.vector.tensor_tensor(
                            ind[:], S[:, 0:NSEG], S[:, 1:P], op=Alu.subtract
                        )
                        nc.tensor.matmul(
                            ps[:], ind[:], xt[:, k * C : (k + 1) * C],
                            start=(ti == 0), stop=(ti == KTILES - 1),
                        )

                out_t = outp.tile([NSEG, C], dt.float32)
                nc.vector.tensor_scalar(out_t[:], ps[:], recip[:], None, op0=Alu.mult)
                nc.sync.dma_start(y_d[r], out_t[:])

    nc.compile()
    return nc


_PROGRAM = None


def _get_program():
    global _PROGRAM
    if _PROGRAM is None:
        _PROGRAM = build_program()
    return _PROGRAM


def kernel(x, x_pos):
    x = np.ascontiguousarray(x, dtype=np.float32)
    x_pos = np.ascontiguousarray(x_pos, dtype=np.int32)
    nc = _get_program()
    in_maps = [
        {"x": x[c * R : (c + 1) * R], "x_pos": x_pos[c * R : (c + 1) * R]}
        for c in range(NCORES)
    ]
    res = run_bass_kernel_spmd(nc, in_maps, list(range(NCORES)))
    y = np.concatenate([res.results[c]["y"] for c in range(NCORES)], axis=0)
    return y.astype(np.float32)


# revision 4
# speedup vs baseline: 1.0472x; 1.0472x over previous
"""Segment mean-pool (LocalPooling1D) Trainium2 Bass kernel.

x [32, 8192, 256] f32, x_pos [32, 65] sorted int32 boundaries -> y [32, 64, 256].
y[b, j] = mean(x[b, x_pos[b,j]:x_pos[b,j+1]]), empty segments -> 0.

Strategy: data-parallel over batch, 4 rows per core on 8 cores. Per 128-token
tile, build the 0/1 segment-indicator [128 tok, 64 seg] on-chip from x_pos
(step-function difference: S[t,j] = (pos[j] <= t), ind = S[:, :64] - S[:, 1:])
and accumulate psum[64, 256] += ind.T @ x_tile on the TensorEngine. Divide by
clamped segment counts at the end.
"""

import os
import sys

import numpy as np

sys.path.insert(0, "/opt/trn_rl_repo")

import concourse.bacc as bacc
import concourse.bass as bass
import concourse.tile as tile
from concourse import mybir
from concourse.bass_utils import run_bass_kernel_spmd

dt = mybir.dt
Alu = mybir.AluOpType

# Problem constants (hardcoded per harness contract).
B, T, C, P = 32, 8192, 256, 65
NSEG = P - 1
NCORES = 8
R = B // NCORES          # batch rows per core
TOK = 128                # tokens per matmul tile (K)
KTILES = T // TOK        # 64 matmul tiles per row

# Tunables (env-overridable for experiments).
CFG = {
    "blk": int(os.environ.get("KB_BLK", "8")),            # token-tiles per x DMA
    "mm_dtype": os.environ.get("KB_MMDT", "f32"),         # f32 | f32r
    "use_act_sign": os.environ.get("KB_ACT", "1") == "1", # S on ScalarE via Sign
    "x_bufs": int(os.environ.get("KB_XBUFS", "4")),
    "ind_bufs": int(os.environ.get("KB_INDBUFS", "8")),
    "psum_bufs": int(os.environ.get("KB_PSUMBUFS", "2")),
}


def build_program(cfg=CFG):
    blk = cfg["blk"]
    nblk = KTILES // blk
    mm_dt = dt.float32r if cfg["mm_dtype"] == "f32r" else dt.float32
    use_act = cfg["use_act_sign"]

    nc = bacc.Bacc("TRN2", target_bir_lowering=False, debug=False)

    x_d = nc.dram_tensor("x", [R, T, C], mm_dt, kind="ExternalInput")
    pos_d = nc.dram_tensor("x_pos", [R, P], dt.int32, kind="ExternalInput")
    y_d = nc.dram_tensor("y", [R, NSEG, C], dt.float32, kind="ExternalOutput")

    with tile.TileContext(nc) as tc:
        with (
            tc.tile_pool(name="const", bufs=1) as constp,
            tc.tile_pool(name="xp", bufs=cfg["x_bufs"]) as xp,
            tc.tile_pool(name="sp", bufs=cfg["ind_bufs"]) as sp,
            tc.tile_pool(name="indp", bufs=cfg["ind_bufs"]) as indp,
            tc.tile_pool(name="smallp", bufs=4) as smallp,
            tc.tile_pool(name="outp", bufs=2) as outp,
            tc.tile_pool(name="psp", bufs=cfg["psum_bufs"], space="PSUM") as psp,
        ):
            # t_global per (partition, tile): val = p + 128*ti.
            tg_i = constp.tile([TOK, KTILES], dt.int32)
            nc.gpsimd.iota(tg_i[:], pattern=[[TOK, KTILES]], base=0, channel_multiplier=1)
            tgf = constp.tile([TOK, KTILES], dt.float32)
            if use_act:
                # sign(t + 0.5 - pos) trick needs the half-offset baked in.
                nc.vector.tensor_scalar(tgf[:], tg_i[:], 0.5, None, op0=Alu.add)
            else:
                nc.vector.tensor_copy(tgf[:], tg_i[:])

            for r in range(R):
                # pos row -> [1, 65] f32, broadcast to [128, 65].
                pos_row = smallp.tile([1, P], dt.int32)
                nc.sync.dma_start(pos_row[:], pos_d[r : r + 1, :])
                posf_row = smallp.tile([1, P], dt.float32)
                nc.vector.tensor_copy(posf_row[:], pos_row[:])
                pos_b = smallp.tile([TOK, P], dt.float32)
                nc.gpsimd.partition_broadcast(pos_b[:], posf_row[:])

                # counts -> 1/max(cnt, 1), partition-major [64, 1].
                pos_lo = smallp.tile([NSEG, 1], dt.int32)
                pos_hi = smallp.tile([NSEG, 1], dt.int32)
                nc.sync.dma_start(pos_lo[:], pos_d[r : r + 1, 0:NSEG].rearrange("one p -> p one"))
                nc.sync.dma_start(pos_hi[:], pos_d[r : r + 1, 1:P].rearrange("one p -> p one"))
                cnt_f = smallp.tile([NSEG, 1], dt.float32)
                nc.vector.tensor_tensor(cnt_f[:], pos_hi[:], pos_lo[:], op=Alu.subtract)
                cntc = smallp.tile([NSEG, 1], dt.float32)
                if use_act:
                    # ind comes out as 2*indicator; fold the 1/2 into the recip.
                    nc.vector.tensor_scalar(cntc[:], cnt_f[:], 1.0, 2.0, op0=Alu.max, op1=Alu.mult)
                else:
                    nc.vector.tensor_scalar(cntc[:], cnt_f[:], 1.0, None, op0=Alu.max)
                recip = smallp.tile([NSEG, 1], dt.float32)
                nc.vector.reciprocal(recip[:], cntc[:])

                ps = psp.tile([NSEG, C], dt.float32)
                xr = x_d[r].rearrange("(b k p) c -> b p k c", k=blk, p=TOK)
                for b in range(nblk):
                    xt = xp.tile([TOK, blk * C], mm_dt)
                    xt_v = xt[:].rearrange("p (k c) -> p k c", k=blk)
                    nc.sync.dma_start(xt_v, xr[b])
                    for k in range(blk):
                        ti = b * blk + k
                        S = sp.tile([TOK, P], dt.float32)
                        if use_act:
                            # S' = sign(t + 0.5 - pos[j]) in {-1, +1} = 2S - 1
                            nc.scalar.activation(
                                S[:], pos_b[:], mybir.ActivationFunctionType.Sign,
                                bias=tgf[:, ti : ti + 1], scale=-1.0,
                            )
                        else:
                            nc.vector.tensor_scalar(
                                S[:], pos_b[:], tgf[:, ti : ti + 1], None, op0=Alu.is_le
                            )
                        ind = indp.tile([TOK, NSEG], mm_dt)
                        nc.vector.tensor_tensor(
                            ind[:], S[:, 0:NSEG], S[:, 1:P], op=Alu.subtract
                        )
                        nc.tensor.matmul(
                            ps[:], ind[:], xt[:, k * C : (k + 1) * C],
                            start=(ti == 0), stop=(ti == KTILES - 1),
                        )

                out_t = outp.tile([NSEG, C], dt.float32)
                nc.vector.tensor_scalar(out_t[:], ps[:], recip[:], None, op0=Alu.mult)
                nc.sync.dma_start(y_d[r], out_t[:])

    nc.compile()
    return nc


_PROGRAM = None


def _get_program():
    global _PROGRAM
    if _PROGRAM is None:
        _PROGRAM = build_program()
    return _PROGRAM


def kernel(x, x_pos):
    x = np.ascontiguousarray(x, dtype=np.float32)
    x_pos = np.ascontiguousarray(x_pos, dtype=np.int32)
    nc = _get_program()
    in_maps = [
        {"x": x[c * R : (c + 1) * R], "x_pos": x_pos[c * R : (c + 1) * R]}
        for c in range(NCORES)
    ]
    res = run_bass_kernel_spmd(nc, in_maps, list(range(NCORES)))
    y = np.concatenate([res.results[c]["y"] for c in range(NCORES)], axis=0)
    return y.astype(np.float32)
